# revision 57
# baseline (speedup 1.0000x reference)
"""Multi-head attention Trainium2 kernel (B=8, N=1024, C=768, H=12, d=64).

Sharding: data-parallel over batch -- core b computes batch element b.

Per-core dataflow (fp16 matmul operands, fp32 PSUM accumulation; fp16 keeps
the PE on its full-clock datapath):
  - host pre-transposes x -> xT [C, N] and all weights -> [in, out] layout,
    folds the 1/sqrt(d) softmax scale into q_w, extends v_w with a zero
    column per head (slot for the softmax-denominator ones trick).
  - Qt = wqT.T @ xT   [C, N]  (transposed layout, heads on partitions)
  - Kt = wkT.T @ xT   [C, N]
  - V' = xT.T @ vwT'  [N, H*65]  (natural layout; col h*65+64 memset to 1.0)
  - per head pair t, token-chunk ch: both heads' transposed scores land in
    one 2-bank PSUM tile st[128, 2, 512]; ONE Exp activation covers the
    pair. The P@V' accumulation runs one m-tile BEHIND the score stream so
    the PE never waits on the just-issued Exp (software pipeline).
    yt'[d'|sum, n] = V'_h.T @ P accumulated over m-tiles; row 64 = colsum
  - per (t, ch): Yt = yt * head_mask[h]^2 / colsum, normalized immediately
    (reciprocal_approx_fast straight off the PSUM colsum rows; partition
    broadcast on GpSimd) so the tail only waits on the final chunk.
  - out = Yt.T @ pwT  [N, C], staged fp16 (ACT-engine eviction), host casts
    back to fp32.
"""

import numpy as np

B, N, C, H, D = 8, 1024, 768, 12, 64
KO = C // 128          # 6 contraction tiles of 128 channels
MT = N // 128          # 8 token tiles
NCH = N // 512         # 2 free-dim chunks of 512
D1 = D + 1             # V' block width per head (64 V cols + 1 ones col)
CV = H * D1            # 780 extended V channels
NCORES = 8

MM_DTYPE = "f16"

_cache = {}


def _build():
    import concourse.bacc as bacc
    import concourse.mybir as mybir
    import concourse.tile as tile

    F32 = mybir.dt.float32
    MMD = {"bf16": mybir.dt.bfloat16, "f16": mybir.dt.float16,
           "f32r": mybir.dt.float32r, "f32": mybir.dt.float32}[MM_DTYPE]
    AF = mybir.ActivationFunctionType

    nc = bacc.Bacc("TRN2", target_bir_lowering=False, debug=False)

    d_xT = nc.dram_tensor("xT", [C, N], MMD, kind="ExternalInput")
    d_wq = nc.dram_tensor("wqT", [C, C], MMD, kind="ExternalInput")
    d_wk = nc.dram_tensor("wkT", [C, C], MMD, kind="ExternalInput")
    d_wv = nc.dram_tensor("vwT", [C, CV], MMD, kind="ExternalInput")
    d_wp = nc.dram_tensor("pwT", [C, C], MMD, kind="ExternalInput")
    d_out = nc.dram_tensor("out", [N, C], MMD, kind="ExternalOutput")

    r_xT = d_xT.ap().rearrange("(ko p) n -> p ko n", p=128)
    r_wq = d_wq.ap().rearrange("(ko p) m -> p ko m", p=128)
    r_wk = d_wk.ap().rearrange("(ko p) m -> p ko m", p=128)
    r_wv = d_wv.ap().rearrange("(ko p) m -> p ko m", p=128)
    r_wp = d_wp.ap().rearrange("(ko p) m -> p ko m", p=128)
    r_out = d_out.ap().rearrange("(mt p) c -> mt p c", p=128)

    with tile.TileContext(nc) as tc:
        with (
            tc.tile_pool(name="xw", bufs=1) as xw,          # xT, vwT, wp (resident)
            tc.tile_pool(name="wq", bufs=3) as wqp,         # streamed weight blocks
            tc.tile_pool(name="wk", bufs=3) as wkp,
            tc.tile_pool(name="qt", bufs=3) as qtp,         # Qt/Kt streamed per pair
            tc.tile_pool(name="kt", bufs=3) as ktp,
            tc.tile_pool(name="vp", bufs=8) as vpp,         # V' all 8 token tiles
            tc.tile_pool(name="yt", bufs=6) as ytp,         # Yt all 6 channel tiles
            tc.tile_pool(name="pp", bufs=9) as ppp,         # P = exp(St), paired
            tc.tile_pool(name="cs", bufs=4) as csp,         # recip rows
            tc.tile_pool(name="bc", bufs=3) as bcp,         # broadcast tiles
            tc.tile_pool(name="ob", bufs=2) as obp,         # output staging
            tc.tile_pool(name="mm", bufs=2, space="PSUM") as mmp,
            tc.tile_pool(name="st", bufs=2, space="PSUM") as stp,
            tc.tile_pool(name="ya", bufs=2, space="PSUM") as yap,
        ):
            # ---- resident tiles. x is tiled per DMA (dependency tracking
            # is tile-granular): ch0 as THREE 2-ko tiles spread over three
            # queues (per-queue bandwidth ~90GB/s limits the head critical
            # path), ch1 as two 3-ko tiles.
            xc0 = [xw.tile([128, 2, 512], MMD, tag=f"xc0{i}", name=f"xc0{i}")
                   for i in range(3)]
            xc1 = [xw.tile([128, 3, 512], MMD, tag=f"xc1{i}", name=f"xc1{i}")
                   for i in range(2)]
            t_wv = xw.tile([128, KO, CV], MMD, tag="wv")
            t_wp = xw.tile([128, KO, C], MMD, tag="wpf")

            def x_ap(ko, col0, w):
                if col0 < 512:
                    return xc0[ko // 2][:, ko % 2, col0:col0 + w]
                c = col0 - 512
                return xc1[ko // 3][:, ko % 3, c:c + w]

            def make_qk(t, dma_engine, dma_engine2=None):
                """DMA the weight blocks for channel tile t and return
                (t_q, t_k, units) where units are deferred emitters, each
                HALF a PSUM accumulation group (3 matmuls; 2nd half also
                evicts)."""
                t_wqb = wqp.tile([128, KO, 128], MMD, tag="wq", name=f"wqb{t}")
                dma_engine.dma_start(
                    out=t_wqb[:], in_=r_wq[:, :, t * 128:(t + 1) * 128]
                )
                t_wkb = wkp.tile([128, KO, 128], MMD, tag="wk", name=f"wkb{t}")
                (dma_engine2 or dma_engine).dma_start(
                    out=t_wkb[:], in_=r_wk[:, :, t * 128:(t + 1) * 128]
                )
                t_q = qtp.tile([128, N], MMD, tag="qt", name=f"q{t}")
                t_k = ktp.tile([128, N], MMD, tag="kt", name=f"k{t}")

                def unit(wsrc, dst, ch, nm):
                    nsl = slice(ch * 512, (ch + 1) * 512)
                    state = {}

                    def part(k0):
                        def emit():
                            if k0 == 0:
                                state["ps"] = mmp.tile([128, 512], F32,
                                                       tag="mm", name=nm)
                            ps = state["ps"]
                            for ko in range(k0, k0 + 3):
                                nc.tensor.matmul(
                                    ps[:], wsrc[:, ko, :],
                                    x_ap(ko, ch * 512, 512),
                                    start=(ko == 0), stop=(ko == KO - 1),
                                )
                            if k0 == 3:
                                nc.vector.tensor_copy(dst[:, nsl], ps[:])
                        return emit

                    return [part(0), part(3)]

                units = []
                units += unit(t_wqb, t_q, 0, f"pq{t}a")
                units += unit(t_wkb, t_k, 0, f"pk{t}a")
                units += unit(t_wqb, t_q, 1, f"pq{t}b")
                units += unit(t_wkb, t_k, 1, f"pk{t}b")
                return t_q, t_k, units

            # ---- head DMAs, spread across idle engine queues so configs
            # issue in parallel: first-needed operands first ----
            t_q, t_k, units0 = make_qk(0, nc.sync, nc.scalar)
            # one DMA per x tile; wq0/wk0 lead the sync/scalar queues so
            # the first projection's operands arrive first; gpsimd carries
            # the remaining operands in consumption order
            nc.sync.dma_start(out=xc0[0][:], in_=r_xT[:, 0:2, 0:512])
            nc.scalar.dma_start(out=xc0[1][:], in_=r_xT[:, 2:4, 0:512])
            nc.gpsimd.dma_start(out=xc0[2][:], in_=r_xT[:, 4:6, 0:512])
            nc.gpsimd.dma_start(out=xc1[0][:], in_=r_xT[:, 0:3, 512:1024])
            nc.gpsimd.dma_start(out=xc1[1][:], in_=r_xT[:, 3:6, 512:1024])
            nc.gpsimd.dma_start(out=t_wv[:, 0:3, :], in_=r_wv[:, 0:3, :])
            nc.gpsimd.dma_start(out=t_wv[:, 3:6, :], in_=r_wv[:, 3:6, :])
            # wp is not needed until the very end; its DMA is issued after
            # pair 0 so it never competes with the critical head loads

            # warm the Exp activation table while the head DMAs stream so
            # the first real exp doesn't pay the ~1.3us table load
            t_warm = xw.tile([1, 2], F32, tag="warm")
            nc.vector.memset(t_warm[0:1, 0:1], 0.0)
            nc.scalar.activation(t_warm[0:1, 1:2], t_warm[0:1, 0:1], AF.Exp)

            # pair-0 ch0 projections run first, q/k parts interleaved in
            # x-chunk arrival order; ch1 projections are woven into the
            # ch0 attention stream
            for idx in (0, 2, 1, 3):
                units0[idx]()
            qk_tiles = {0: (t_q, t_k)}
            created = 0
            pend = []  # (need_by_tile_idx, deferred emitter)

            # ---- V' projection units (weavable): V'[n, cv] = xT.T @ vwT ----
            t_v = [vpp.tile([128, CV], MMD, tag="v", name=f"v{mt}")
                   for mt in range(MT)]
            vch = [(0, 390), (390, 390)]

            def v_unit(mt):
                tv = t_v[mt]
                state = {}

                def part_a():
                    ps = mmp.tile([128, 512], F32, tag="mm", name=f"v{mt}a")
                    state["ps"] = ps
                    for ko in range(KO):
                        nc.tensor.matmul(
                            ps[:, :390], x_ap(ko, mt * 128, 128),
                            t_wv[:, ko, 0:390],
                            start=(ko == 0), stop=(ko == KO - 1),
                        )
                    nc.vector.tensor_copy(tv[:, 0:390], ps[:, :390])

                def part_b():
                    ps = mmp.tile([128, 512], F32, tag="mm", name=f"v{mt}b")
                    for ko in range(KO):
                        nc.tensor.matmul(
                            ps[:, :390], x_ap(ko, mt * 128, 128),
                            t_wv[:, ko, 390:780],
                            start=(ko == 0), stop=(ko == KO - 1),
                        )
                    nc.vector.tensor_copy(tv[:, 390:780], ps[:, :390])
                    ones_cols = tv[:].rearrange(
                        "p (h e) -> p h e", e=D1)[:, :, D:D + 1]
                    nc.vector.memset(ones_cols, 1.0)

                return [part_a, part_b]

            # all V' units weave into pair 0's attention stream; pair 0's
            # ch0 PV wave is deferred until after its full score wave, by
            # which time the early V' tiles have landed. ch1's Q/K
            # projections follow the V' units (x ch1 lands late).
            # pk0b first (K ch1 is read by every score m-tile >= 4), then
            # V'[0] (needed by the first PV), then pq0b (ch1 queries),
            # then the remaining V' units
            pend.extend((0, u) for u in units0[6:8])
            pend.extend((0, u) for u in v_unit(0))
            pend.extend((0, u) for u in units0[4:6])
            for mt in range(1, MT):
                pend.extend((0, u) for u in v_unit(mt))

            t_yt = [ytp.tile([128, N], MMD, tag="yt", name=f"yt{i}")
                    for i in range(KO)]

            # ---- output projection units: out[n, c] = Yt.T @ pwT; yt
            # stationary shared by both output chunks (halves the
            # LDWEIGHTS). mt 4-7 (ch1 tokens, normalized first by the last
            # pair) weave into the last pair's ch0 stream; mt 0-3 run at
            # the tail. ----
            def oproj_unit(mt, poolA, dma_eng):
                t_o = obp.tile([128, C], MMD, tag="ob", name=f"ob{mt}")
                msl = slice(mt * 128, (mt + 1) * 128)
                state = {}

                def half_a():
                    psA = poolA.tile([128, 512], F32, tag=poolA is stp and "st"
                                     or "mm", name=f"poA{mt}")
                    state["psA"] = psA
                    for t_ in range(KO):
                        nc.tensor.matmul(
                            psA[:], t_yt[t_][:, msl], t_wp[:, t_, 0:512],
                            start=(t_ == 0), stop=(t_ == KO - 1),
                        )
                    nc.vector.tensor_copy(t_o[:, 0:512], state["psA"][:])

                def half_b():
                    psB = mmp.tile([128, 512], F32, tag="mm", name=f"poB{mt}")
                    for t_ in range(KO):
                        nc.tensor.matmul(
                            psB[:, 0:256], t_yt[t_][:, msl],
                            t_wp[:, t_, 512:768],
                            start=(t_ == 0), stop=(t_ == KO - 1),
                        )
                    nc.vector.tensor_copy(t_o[:, 512:768], psB[:, 0:256])
                    dma_eng.dma_start(out=r_out[mt, :, :], in_=t_o[:])

                return [half_a, half_b]

            # ---- per channel-tile: attention pair with PV software-pipelined
            # one m-tile behind the score stream; projection half-groups for
            # later pairs woven in as PE filler ----

            for t in range(KO):
                cap = 1 if t == 0 else min(t + 2, KO - 1)
                while created < cap:
                    created += 1
                    q_, k_, us = make_qk(created, nc.gpsimd)
                    qk_tiles[created] = (q_, k_)
                    pend.extend((created, u) for u in us)
                if t == 3:
                    nc.gpsimd.dma_start(out=t_wp[:], in_=r_wp[:])

                # the last pair runs ch1 first so ch0 (whose tokens the
                # output projection consumes first) is normalized last but
                # the projection starts on ch1 tokens immediately
                ch_order = [1, 0] if t == KO - 1 else [0, 1]
                for ch in ch_order:
                    if t == KO - 1 and ch == 0:
                        # ch1 is fully normalized now: weave its output
                        # projection into this chunk's attention stream
                        for mt_ in (4, 5, 6, 7):
                            pend.extend(
                                (t, u) for u in oproj_unit(mt_, mmp, nc.sync))
                    nsl = slice(ch * 512, (ch + 1) * 512)
                    yt0 = yap.tile([D1, 512], F32, tag="ya", name=f"ya{t}{ch}0")
                    yt1 = yap.tile([D1, 512], F32, tag="ya", name=f"ya{t}{ch}1")
                    p_tiles = [None] * MT
                    # pair-0 ch0 runs its whole score wave before any PV
                    # (the V' tiles PV needs are still streaming in); the
                    # steady state runs PV one m-tile behind the scores
                    first = (t == 0 and ch == ch_order[0])
                    lag = MT if first else 3
                    for step in range(MT + lag):
                        if step < MT:
                            mt = step
                            msl = slice(mt * 128, (mt + 1) * 128)
                            st = stp.tile([128, 2, 512], F32, tag="st",
                                          name=f"st{t}{ch}{mt}")
                            nc.tensor.matmul(
                                st[:, 0, :], t_k[0:64, msl], t_q[0:64, nsl],
                                start=True, stop=True, tile_position=(0, 0),
                            )
                            nc.tensor.matmul(
                                st[:, 1, :], t_k[64:128, msl], t_q[64:128, nsl],
                                start=True, stop=True, tile_position=(64, 0),
                            )
                            p = ppp.tile([128, 2, 512], MMD, tag="p",
                                         name=f"p{t}{ch}{mt}")
                            nc.scalar.activation(p[:], st[:], AF.Exp)
                            p_tiles[mt] = p
                            # pk0b lands here: K ch1 must be complete
                            # before the step-4 score reads it
                            if first and step in (2, 3) and pend:
                                pend.pop(0)[1]()
                        if step >= lag:
                            mt = step - lag
                            # during pair-0's PV wave the remaining setup
                            # units pop just-in-time BEFORE the PV that
                            # consumes them
                            if first:
                                for _ in range(4):
                                    if pend:
                                        pend.pop(0)[1]()
                            p = p_tiles[mt]
                            nc.tensor.matmul(
                                yt0[:], t_v[mt][:, (2 * t) * D1:(2 * t + 1) * D1],
                                p[:, 0, :], start=(mt == 0), stop=(mt == MT - 1),
                            )
                            nc.tensor.matmul(
                                yt1[:],
                                t_v[mt][:, (2 * t + 1) * D1:(2 * t + 2) * D1],
                                p[:, 1, :], start=(mt == 0), stop=(mt == MT - 1),
                            )
                        # weave projection part-groups into the stream;
                        # drain faster under backlog so pair boundaries
                        # don't inherit a burst of forced evictions
                        if not first:
                            for _ in range(2 if len(pend) > 12 else 1):
                                if pend:
                                    pend.pop(0)[1]()
                    # evict + normalize this chunk on DVE/GpSimd while the
                    # PE streams on. Mid-kernel the PSUM accumulators are
                    # drained slot-by-slot (cs+evict per head) so the next
                    # chunk's PV can claim each ya slot ASAP; on the last
                    # pair the reciprocal chain leads instead because the
                    # output projection waits on the normalized result.
                    t_cs = csp.tile([1, 2, 512], F32, tag="cs", name=f"cs{t}{ch}")
                    t_rc = csp.tile([1, 2, 512], F32, tag="rc", name=f"rc{t}{ch}")
                    if t < KO - 1:
                        nc.vector.tensor_copy(t_cs[0:1, 0, :], yt0[D:D1, :])
                        nc.vector.tensor_copy(t_yt[t][0:64, nsl], yt0[0:D, :])
                        nc.vector.tensor_copy(t_cs[0:1, 1, :], yt1[D:D1, :])
                        nc.vector.tensor_copy(t_yt[t][64:128, nsl], yt1[0:D, :])
                        nc.vector.reciprocal_approx_fast(t_rc[0:1, 0, :],
                                                         t_cs[0:1, 0, :])
                        nc.vector.reciprocal_approx_fast(t_rc[0:1, 1, :],
                                                         t_cs[0:1, 1, :])
                    else:
                        nc.vector.tensor_copy(t_cs[0:1, 0, :], yt0[D:D1, :])
                        nc.vector.tensor_copy(t_cs[0:1, 1, :], yt1[D:D1, :])
                        nc.vector.reciprocal_approx_fast(t_rc[0:1, 0, :],
                                                         t_cs[0:1, 0, :])
                        nc.vector.reciprocal_approx_fast(t_rc[0:1, 1, :],
                                                         t_cs[0:1, 1, :])
                        nc.vector.tensor_copy(t_yt[t][0:64, nsl], yt0[0:D, :])
                        nc.vector.tensor_copy(t_yt[t][64:128, nsl], yt1[0:D, :])
                    for hp in range(2):
                        psl = slice(hp * 64, hp * 64 + 64)
                        t_bc = bcp.tile([128, 512], F32, tag="bc",
                                        name=f"bc{t}{ch}{hp}")
                        nc.gpsimd.partition_broadcast(
                            t_bc[:], t_rc[0:1, hp, :]
                        )
                        nc.vector.tensor_mul(
                            t_yt[t][psl, nsl], t_yt[t][psl, nsl], t_bc[psl, :]
                        )
                    # anything still pending that this pair needs must land
                    # before the next chunk reads it
                    due = [pu for pu in pend if pu[0] <= t]
                    if due:
                        for i, u in due:
                            u()
                        pend = [pu for pu in pend if pu[0] > t]
                # next pair's projections must be complete before it starts
                for i, u in [pu for pu in pend if pu[0] == t + 1]:
                    u()
                pend = [pu for pu in pend if pu[0] != t + 1]
                if t + 1 < KO:
                    t_q, t_k = qk_tiles[t + 1]

            # ---- tail: output projection for the ch0 token tiles ----
            for mt in (0, 1, 2, 3):
                eng = nc.gpsimd if mt % 2 else nc.sync
                for u in oproj_unit(mt, stp, eng):
                    u()

    nc.compile()
    return nc


def _prep_inputs(x, head_mask, q_w, k_w, v_w, proj_w):
    import ml_dtypes

    mmnp = {"bf16": ml_dtypes.bfloat16, "f16": np.float16,
            "f32r": np.float32, "f32": np.float32}[MM_DTYPE]
    scale = np.float32(D ** -0.5)
    wqT = np.ascontiguousarray((q_w * scale).T).astype(mmnp)
    wkT = np.ascontiguousarray(k_w.T).astype(mmnp)
    vwT0 = np.zeros((C, CV), np.float32)
    vT = v_w.T.astype(np.float32)
    for h in range(H):
        vwT0[:, h * D1:h * D1 + D] = vT[:, h * D:(h + 1) * D]
    pwT = np.ascontiguousarray(proj_w.T).astype(mmnp)
    in_maps = []
    for b in range(NCORES):
        xT = np.ascontiguousarray(x[b].T).astype(mmnp)
        # fold head_mask^2 into this core's V weights (ones cols stay 0->1)
        vwT = vwT0.copy()
        for h in range(H):
            vwT[:, h * D1:h * D1 + D] *= head_mask[b, h] ** 2
        in_maps.append(
            {"xT": xT, "wqT": wqT, "wkT": wkT, "vwT": vwT.astype(mmnp),
             "pwT": pwT}
        )
    return in_maps


def _run(inputs, trace=False):
    from concourse.bass_utils import run_bass_kernel_spmd

    x = np.asarray(inputs["x"], np.float32)
    head_mask = np.asarray(inputs["head_mask"], np.float32)
    in_maps = _prep_inputs(
        x,
        head_mask,
        np.asarray(inputs["q_w"], np.float32),
        np.asarray(inputs["k_w"], np.float32),
        np.asarray(inputs["v_w"], np.float32),
        np.asarray(inputs["proj_w"], np.float32),
    )
    # biases are zero by construction of this problem (spec fill=zeros);
    # q_b/k_b/v_b/proj_b are validated and otherwise unused.
    for name in ("q_b", "k_b", "v_b", "proj_b"):
        bias = np.asarray(inputs[name])
        if np.abs(bias).max() > 0:
            raise NotImplementedError(f"nonzero {name} not supported")

    if "nc" not in _cache:
        _cache["nc"] = _build()
    nc = _cache["nc"]
    res = run_bass_kernel_spmd(
        nc, in_maps, core_ids=list(range(NCORES)), trace=trace
    )
    out = np.stack([res.results[b]["out"] for b in range(NCORES)], axis=0)
    return out.astype(np.float32), res


def kernel(**inputs):
    out, _ = _run(inputs, trace=False)
    return out


# revision 58
# speedup vs baseline: 1.0212x; 1.0212x over previous
"""Multi-head attention Trainium2 kernel (B=8, N=1024, C=768, H=12, d=64).

Sharding: data-parallel over batch -- core b computes batch element b.

Per-core dataflow (fp16 matmul operands, fp32 PSUM accumulation; fp16 keeps
the PE on its full-clock datapath):
  - host pre-transposes x -> xT [C, N] and all weights -> [in, out] layout,
    folds the 1/sqrt(d) softmax scale into q_w, extends v_w with a zero
    column per head (slot for the softmax-denominator ones trick).
  - Qt = wqT.T @ xT   [C, N]  (transposed layout, heads on partitions)
  - Kt = wkT.T @ xT   [C, N]
  - V' = xT.T @ vwT'  [N, H*65]  (natural layout; col h*65+64 memset to 1.0)
  - per head pair t, token-chunk ch: both heads' transposed scores land in
    one 2-bank PSUM tile st[128, 2, 512]; ONE Exp activation covers the
    pair. The P@V' accumulation runs one m-tile BEHIND the score stream so
    the PE never waits on the just-issued Exp (software pipeline).
    yt'[d'|sum, n] = V'_h.T @ P accumulated over m-tiles; row 64 = colsum
  - per (t, ch): Yt = yt * head_mask[h]^2 / colsum, normalized immediately
    (reciprocal_approx_fast straight off the PSUM colsum rows; partition
    broadcast on GpSimd) so the tail only waits on the final chunk.
  - out = Yt.T @ pwT  [N, C], staged fp16 (ACT-engine eviction), host casts
    back to fp32.
"""

import numpy as np

B, N, C, H, D = 8, 1024, 768, 12, 64
KO = C // 128          # 6 contraction tiles of 128 channels
MT = N // 128          # 8 token tiles
NCH = N // 512         # 2 free-dim chunks of 512
D1 = D + 1             # V' block width per head (64 V cols + 1 ones col)
CV = H * D1            # 780 extended V channels
NCORES = 8

MM_DTYPE = "f16"

_cache = {}


def _build():
    import concourse.bacc as bacc
    import concourse.mybir as mybir
    import concourse.tile as tile

    F32 = mybir.dt.float32
    MMD = {"bf16": mybir.dt.bfloat16, "f16": mybir.dt.float16,
           "f32r": mybir.dt.float32r, "f32": mybir.dt.float32}[MM_DTYPE]
    AF = mybir.ActivationFunctionType

    nc = bacc.Bacc("TRN2", target_bir_lowering=False, debug=False)

    d_xT = nc.dram_tensor("xT", [C, N], MMD, kind="ExternalInput")
    d_wq = nc.dram_tensor("wqT", [C, C], MMD, kind="ExternalInput")
    d_wk = nc.dram_tensor("wkT", [C, C], MMD, kind="ExternalInput")
    d_wv = nc.dram_tensor("vwT", [C, CV], MMD, kind="ExternalInput")
    d_wp = nc.dram_tensor("pwT", [C, C], MMD, kind="ExternalInput")
    d_out = nc.dram_tensor("out", [N, C], MMD, kind="ExternalOutput")

    r_xT = d_xT.ap().rearrange("(ko p) n -> p ko n", p=128)
    r_wq = d_wq.ap().rearrange("(ko p) m -> p ko m", p=128)
    r_wk = d_wk.ap().rearrange("(ko p) m -> p ko m", p=128)
    r_wv = d_wv.ap().rearrange("(ko p) m -> p ko m", p=128)
    r_wp = d_wp.ap().rearrange("(ko p) m -> p ko m", p=128)
    r_out = d_out.ap().rearrange("(mt p) c -> mt p c", p=128)

    with tile.TileContext(nc) as tc:
        with (
            tc.tile_pool(name="xw", bufs=1) as xw,          # xT, vwT, wp (resident)
            tc.tile_pool(name="wq", bufs=3) as wqp,         # streamed weight blocks
            tc.tile_pool(name="wk", bufs=3) as wkp,
            tc.tile_pool(name="qt", bufs=3) as qtp,         # Qt/Kt streamed per pair
            tc.tile_pool(name="kt", bufs=3) as ktp,
            tc.tile_pool(name="vp", bufs=8) as vpp,         # V' all 8 token tiles
            tc.tile_pool(name="yt", bufs=6) as ytp,         # Yt all 6 channel tiles
            tc.tile_pool(name="pp", bufs=9) as ppp,         # P = exp(St), paired
            tc.tile_pool(name="cs", bufs=4) as csp,         # recip rows
            tc.tile_pool(name="bc", bufs=3) as bcp,         # broadcast tiles
            tc.tile_pool(name="ob", bufs=2) as obp,         # output staging
            tc.tile_pool(name="mm", bufs=2, space="PSUM") as mmp,
            tc.tile_pool(name="st", bufs=2, space="PSUM") as stp,
            tc.tile_pool(name="ya", bufs=2, space="PSUM") as yap,
        ):
            # ---- resident tiles. x is tiled per DMA (dependency tracking
            # is tile-granular): ch0 as THREE 2-ko tiles spread over three
            # queues (per-queue bandwidth ~90GB/s limits the head critical
            # path), ch1 as two 3-ko tiles.
            xc0 = [xw.tile([128, 2, 512], MMD, tag=f"xc0{i}", name=f"xc0{i}")
                   for i in range(3)]
            xc1 = [xw.tile([128, 3, 512], MMD, tag=f"xc1{i}", name=f"xc1{i}")
                   for i in range(2)]
            t_wv = xw.tile([128, KO, CV], MMD, tag="wv")
            t_wp = xw.tile([128, KO, C], MMD, tag="wpf")

            def x_ap(ko, col0, w):
                if col0 < 512:
                    return xc0[ko // 2][:, ko % 2, col0:col0 + w]
                c = col0 - 512
                return xc1[ko // 3][:, ko % 3, c:c + w]

            def make_qk(t, dma_engine, dma_engine2=None):
                """DMA the weight blocks for channel tile t and return
                (t_q, t_k, units) where units are deferred emitters, each
                HALF a PSUM accumulation group (3 matmuls; 2nd half also
                evicts)."""
                t_wqb = wqp.tile([128, KO, 128], MMD, tag="wq", name=f"wqb{t}")
                dma_engine.dma_start(
                    out=t_wqb[:], in_=r_wq[:, :, t * 128:(t + 1) * 128]
                )
                t_wkb = wkp.tile([128, KO, 128], MMD, tag="wk", name=f"wkb{t}")
                (dma_engine2 or dma_engine).dma_start(
                    out=t_wkb[:], in_=r_wk[:, :, t * 128:(t + 1) * 128]
                )
                t_q = qtp.tile([128, N], MMD, tag="qt", name=f"q{t}")
                t_k = ktp.tile([128, N], MMD, tag="kt", name=f"k{t}")

                def unit(wsrc, dst, ch, nm):
                    nsl = slice(ch * 512, (ch + 1) * 512)
                    state = {}

                    def part(kos, is_first, is_last):
                        def emit():
                            if is_first:
                                state["ps"] = mmp.tile([128, 512], F32,
                                                       tag="mm", name=nm)
                            ps = state["ps"]
                            for j, ko in enumerate(kos):
                                nc.tensor.matmul(
                                    ps[:], wsrc[:, ko, :],
                                    x_ap(ko, ch * 512, 512),
                                    start=(is_first and j == 0),
                                    stop=(is_last and j == len(kos) - 1),
                                )
                            if is_last:
                                nc.vector.tensor_copy(dst[:, nsl], ps[:])
                        return emit

                    if t == 0:
                        # pair 0's x chunks stream in during the kernel
                        # head: contract in arrival order
                        return [part((4, 5, 0), True, False),
                                part((1, 2, 3), False, True)]
                    return [part((0, 1, 2), True, False),
                            part((3, 4, 5), False, True)]

                units = []
                units += unit(t_wqb, t_q, 0, f"pq{t}a")
                units += unit(t_wkb, t_k, 0, f"pk{t}a")
                units += unit(t_wqb, t_q, 1, f"pq{t}b")
                units += unit(t_wkb, t_k, 1, f"pk{t}b")
                return t_q, t_k, units

            # ---- head DMAs, spread across idle engine queues so configs
            # issue in parallel: first-needed operands first ----
            t_q, t_k, units0 = make_qk(0, nc.sync, nc.scalar)
            # one DMA per x tile; wq0/wk0 lead the sync/scalar queues so
            # the first projection's operands arrive first; gpsimd carries
            # the remaining operands in consumption order
            nc.sync.dma_start(out=xc0[0][:], in_=r_xT[:, 0:2, 0:512])
            nc.scalar.dma_start(out=xc0[1][:], in_=r_xT[:, 2:4, 0:512])
            nc.gpsimd.dma_start(out=xc0[2][:], in_=r_xT[:, 4:6, 0:512])
            nc.gpsimd.dma_start(out=xc1[0][:], in_=r_xT[:, 0:3, 512:1024])
            nc.gpsimd.dma_start(out=xc1[1][:], in_=r_xT[:, 3:6, 512:1024])
            nc.gpsimd.dma_start(out=t_wv[:, 0:3, :], in_=r_wv[:, 0:3, :])
            nc.gpsimd.dma_start(out=t_wv[:, 3:6, :], in_=r_wv[:, 3:6, :])
            # wp is not needed until the very end; its DMA is issued after
            # pair 0 so it never competes with the critical head loads

            # warm the Exp activation table while the head DMAs stream so
            # the first real exp doesn't pay the ~1.3us table load
            t_warm = xw.tile([1, 2], F32, tag="warm")
            nc.vector.memset(t_warm[0:1, 0:1], 0.0)
            nc.scalar.activation(t_warm[0:1, 1:2], t_warm[0:1, 0:1], AF.Exp)

            # pair-0 ch0 projections run first, q/k parts interleaved in
            # x-chunk arrival order; ch1 projections are woven into the
            # ch0 attention stream
            for idx in (0, 2, 1, 3):
                units0[idx]()
            qk_tiles = {0: (t_q, t_k)}
            created = 0
            pend = []  # (need_by_tile_idx, deferred emitter)

            # ---- V' projection units (weavable): V'[n, cv] = xT.T @ vwT ----
            t_v = [vpp.tile([128, CV], MMD, tag="v", name=f"v{mt}")
                   for mt in range(MT)]
            vch = [(0, 390), (390, 390)]

            def v_unit(mt):
                tv = t_v[mt]
                state = {}

                def part_a():
                    ps = mmp.tile([128, 512], F32, tag="mm", name=f"v{mt}a")
                    state["ps"] = ps
                    for ko in range(KO):
                        nc.tensor.matmul(
                            ps[:, :390], x_ap(ko, mt * 128, 128),
                            t_wv[:, ko, 0:390],
                            start=(ko == 0), stop=(ko == KO - 1),
                        )
                    nc.vector.tensor_copy(tv[:, 0:390], ps[:, :390])

                def part_b():
                    ps = mmp.tile([128, 512], F32, tag="mm", name=f"v{mt}b")
                    for ko in range(KO):
                        nc.tensor.matmul(
                            ps[:, :390], x_ap(ko, mt * 128, 128),
                            t_wv[:, ko, 390:780],
                            start=(ko == 0), stop=(ko == KO - 1),
                        )
                    nc.vector.tensor_copy(tv[:, 390:780], ps[:, :390])
                    ones_cols = tv[:].rearrange(
                        "p (h e) -> p h e", e=D1)[:, :, D:D + 1]
                    nc.vector.memset(ones_cols, 1.0)

                return [part_a, part_b]

            # all V' units weave into pair 0's attention stream; pair 0's
            # ch0 PV wave is deferred until after its full score wave, by
            # which time the early V' tiles have landed. ch1's Q/K
            # projections follow the V' units (x ch1 lands late).
            # pk0b first (K ch1 is read by every score m-tile >= 4), then
            # V'[0] (needed by the first PV), then pq0b (ch1 queries),
            # then the remaining V' units
            pend.extend((0, u) for u in units0[6:8])
            pend.extend((0, u) for u in v_unit(0))
            pend.extend((0, u) for u in units0[4:6])
            for mt in range(1, MT):
                pend.extend((0, u) for u in v_unit(mt))

            t_yt = [ytp.tile([128, N], MMD, tag="yt", name=f"yt{i}")
                    for i in range(KO)]

            # ---- output projection units: out[n, c] = Yt.T @ pwT; yt
            # stationary shared by both output chunks (halves the
            # LDWEIGHTS). mt 4-7 (ch1 tokens, normalized first by the last
            # pair) weave into the last pair's ch0 stream; mt 0-3 run at
            # the tail. ----
            def oproj_unit(mt, poolA, dma_eng):
                t_o = obp.tile([128, C], MMD, tag="ob", name=f"ob{mt}")
                msl = slice(mt * 128, (mt + 1) * 128)
                state = {}

                def half_a():
                    psA = poolA.tile([128, 512], F32, tag=poolA is stp and "st"
                                     or "mm", name=f"poA{mt}")
                    state["psA"] = psA
                    for t_ in range(KO):
                        nc.tensor.matmul(
                            psA[:], t_yt[t_][:, msl], t_wp[:, t_, 0:512],
                            start=(t_ == 0), stop=(t_ == KO - 1),
                        )
                    nc.vector.tensor_copy(t_o[:, 0:512], state["psA"][:])

                def half_b():
                    psB = mmp.tile([128, 512], F32, tag="mm", name=f"poB{mt}")
                    for t_ in range(KO):
                        nc.tensor.matmul(
                            psB[:, 0:256], t_yt[t_][:, msl],
                            t_wp[:, t_, 512:768],
                            start=(t_ == 0), stop=(t_ == KO - 1),
                        )
                    nc.vector.tensor_copy(t_o[:, 512:768], psB[:, 0:256])
                    dma_eng.dma_start(out=r_out[mt, :, :], in_=t_o[:])

                return [half_a, half_b]

            # ---- per channel-tile: attention pair with PV software-pipelined
            # one m-tile behind the score stream; projection half-groups for
            # later pairs woven in as PE filler ----

            for t in range(KO):
                cap = 1 if t == 0 else min(t + 2, KO - 1)
                while created < cap:
                    created += 1
                    q_, k_, us = make_qk(created, nc.gpsimd)
                    qk_tiles[created] = (q_, k_)
                    pend.extend((created, u) for u in us)
                if t == 3:
                    nc.gpsimd.dma_start(out=t_wp[:], in_=r_wp[:])

                # the last pair runs ch1 first so ch0 (whose tokens the
                # output projection consumes first) is normalized last but
                # the projection starts on ch1 tokens immediately
                ch_order = [1, 0] if t == KO - 1 else [0, 1]
                for ch in ch_order:
                    if t == KO - 1 and ch == 0:
                        # ch1 is fully normalized now: weave its output
                        # projection into this chunk's attention stream
                        for mt_ in (4, 5, 6, 7):
                            pend.extend(
                                (t, u) for u in oproj_unit(mt_, mmp, nc.sync))
                    nsl = slice(ch * 512, (ch + 1) * 512)
                    yt0 = yap.tile([D1, 512], F32, tag="ya", name=f"ya{t}{ch}0")
                    yt1 = yap.tile([D1, 512], F32, tag="ya", name=f"ya{t}{ch}1")
                    p_tiles = [None] * MT
                    # pair-0 ch0 runs its whole score wave before any PV
                    # (the V' tiles PV needs are still streaming in); the
                    # steady state runs PV one m-tile behind the scores
                    first = (t == 0 and ch == ch_order[0])
                    lag = MT if first else 2
                    for step in range(MT + lag):
                        if step < MT:
                            mt = step
                            msl = slice(mt * 128, (mt + 1) * 128)
                            st = stp.tile([128, 2, 512], F32, tag="st",
                                          name=f"st{t}{ch}{mt}")
                            nc.tensor.matmul(
                                st[:, 0, :], t_k[0:64, msl], t_q[0:64, nsl],
                                start=True, stop=True, tile_position=(0, 0),
                            )
                            nc.tensor.matmul(
                                st[:, 1, :], t_k[64:128, msl], t_q[64:128, nsl],
                                start=True, stop=True, tile_position=(64, 0),
                            )
                            p = ppp.tile([128, 2, 512], MMD, tag="p",
                                         name=f"p{t}{ch}{mt}")
                            nc.scalar.activation(p[:], st[:], AF.Exp)
                            p_tiles[mt] = p
                            # pk0b lands here: K ch1 must be complete
                            # before the step-4 score reads it
                            if first and step in (2, 3) and pend:
                                pend.pop(0)[1]()
                        if step >= lag:
                            mt = step - lag
                            # during pair-0's PV wave the remaining setup
                            # units pop just-in-time BEFORE the PV that
                            # consumes them
                            if first:
                                for _ in range(4):
                                    if pend:
                                        pend.pop(0)[1]()
                            p = p_tiles[mt]
                            nc.tensor.matmul(
                                yt0[:], t_v[mt][:, (2 * t) * D1:(2 * t + 1) * D1],
                                p[:, 0, :], start=(mt == 0), stop=(mt == MT - 1),
                            )
                            nc.tensor.matmul(
                                yt1[:],
                                t_v[mt][:, (2 * t + 1) * D1:(2 * t + 2) * D1],
                                p[:, 1, :], start=(mt == 0), stop=(mt == MT - 1),
                            )
                        # weave projection part-groups into the stream;
                        # drain faster under backlog so pair boundaries
                        # don't inherit a burst of forced evictions
                        if not first:
                            for _ in range(2 if len(pend) > 12 else 1):
                                if pend:
                                    pend.pop(0)[1]()
                    # evict + normalize this chunk on DVE/GpSimd while the
                    # PE streams on. Mid-kernel the PSUM accumulators are
                    # drained slot-by-slot (cs+evict per head) so the next
                    # chunk's PV can claim each ya slot ASAP; on the last
                    # pair the reciprocal chain leads instead because the
                    # output projection waits on the normalized result.
                    t_cs = csp.tile([1, 2, 512], F32, tag="cs", name=f"cs{t}{ch}")
                    t_rc = csp.tile([1, 2, 512], F32, tag="rc", name=f"rc{t}{ch}")
                    if t < KO - 1:
                        nc.vector.tensor_copy(t_cs[0:1, 0, :], yt0[D:D1, :])
                        nc.vector.tensor_copy(t_yt[t][0:64, nsl], yt0[0:D, :])
                        nc.vector.tensor_copy(t_cs[0:1, 1, :], yt1[D:D1, :])
                        nc.vector.tensor_copy(t_yt[t][64:128, nsl], yt1[0:D, :])
                        nc.vector.reciprocal_approx_fast(t_rc[0:1, 0, :],
                                                         t_cs[0:1, 0, :])
                        nc.vector.reciprocal_approx_fast(t_rc[0:1, 1, :],
                                                         t_cs[0:1, 1, :])
                    else:
                        nc.vector.tensor_copy(t_cs[0:1, 0, :], yt0[D:D1, :])
                        nc.vector.tensor_copy(t_cs[0:1, 1, :], yt1[D:D1, :])
                        nc.vector.reciprocal_approx_fast(t_rc[0:1, 0, :],
                                                         t_cs[0:1, 0, :])
                        nc.vector.reciprocal_approx_fast(t_rc[0:1, 1, :],
                                                         t_cs[0:1, 1, :])
                        nc.vector.tensor_copy(t_yt[t][0:64, nsl], yt0[0:D, :])
                        nc.vector.tensor_copy(t_yt[t][64:128, nsl], yt1[0:D, :])
                    for hp in range(2):
                        psl = slice(hp * 64, hp * 64 + 64)
                        t_bc = bcp.tile([128, 512], F32, tag="bc",
                                        name=f"bc{t}{ch}{hp}")
                        nc.gpsimd.partition_broadcast(
                            t_bc[:], t_rc[0:1, hp, :]
                        )
                        nc.vector.tensor_mul(
                            t_yt[t][psl, nsl], t_yt[t][psl, nsl], t_bc[psl, :]
                        )
                    # anything still pending that this pair needs must land
                    # before the next chunk reads it
                    due = [pu for pu in pend if pu[0] <= t]
                    if due:
                        for i, u in due:
                            u()
                        pend = [pu for pu in pend if pu[0] > t]
                # next pair's projections must be complete before it starts
                for i, u in [pu for pu in pend if pu[0] == t + 1]:
                    u()
                pend = [pu for pu in pend if pu[0] != t + 1]
                if t + 1 < KO:
                    t_q, t_k = qk_tiles[t + 1]

            # ---- tail: output projection for the ch0 token tiles ----
            for mt in (0, 1, 2, 3):
                eng = nc.gpsimd if mt % 2 else nc.sync
                for u in oproj_unit(mt, stp, eng):
                    u()

    nc.compile()
    return nc


def _prep_inputs(x, head_mask, q_w, k_w, v_w, proj_w):
    import ml_dtypes

    mmnp = {"bf16": ml_dtypes.bfloat16, "f16": np.float16,
            "f32r": np.float32, "f32": np.float32}[MM_DTYPE]
    scale = np.float32(D ** -0.5)
    wqT = np.ascontiguousarray((q_w * scale).T).astype(mmnp)
    wkT = np.ascontiguousarray(k_w.T).astype(mmnp)
    vwT0 = np.zeros((C, CV), np.float32)
    vT = v_w.T.astype(np.float32)
    for h in range(H):
        vwT0[:, h * D1:h * D1 + D] = vT[:, h * D:(h + 1) * D]
    pwT = np.ascontiguousarray(proj_w.T).astype(mmnp)
    in_maps = []
    for b in range(NCORES):
        xT = np.ascontiguousarray(x[b].T).astype(mmnp)
        # fold head_mask^2 into this core's V weights (ones cols stay 0->1)
        vwT = vwT0.copy()
        for h in range(H):
            vwT[:, h * D1:h * D1 + D] *= head_mask[b, h] ** 2
        in_maps.append(
            {"xT": xT, "wqT": wqT, "wkT": wkT, "vwT": vwT.astype(mmnp),
             "pwT": pwT}
        )
    return in_maps


def _run(inputs, trace=False):
    from concourse.bass_utils import run_bass_kernel_spmd

    x = np.asarray(inputs["x"], np.float32)
    head_mask = np.asarray(inputs["head_mask"], np.float32)
    in_maps = _prep_inputs(
        x,
        head_mask,
        np.asarray(inputs["q_w"], np.float32),
        np.asarray(inputs["k_w"], np.float32),
        np.asarray(inputs["v_w"], np.float32),
        np.asarray(inputs["proj_w"], np.float32),
    )
    # biases are zero by construction of this problem (spec fill=zeros);
    # q_b/k_b/v_b/proj_b are validated and otherwise unused.
    for name in ("q_b", "k_b", "v_b", "proj_b"):
        bias = np.asarray(inputs[name])
        if np.abs(bias).max() > 0:
            raise NotImplementedError(f"nonzero {name} not supported")

    if "nc" not in _cache:
        _cache["nc"] = _build()
    nc = _cache["nc"]
    res = run_bass_kernel_spmd(
        nc, in_maps, core_ids=list(range(NCORES)), trace=trace
    )
    out = np.stack([res.results[b]["out"] for b in range(NCORES)], axis=0)
    return out.astype(np.float32), res


def kernel(**inputs):
    out, _ = _run(inputs, trace=False)
    return out
